# revision 21
# baseline (speedup 1.0000x reference)
"""BiMamba4TS Trainium2 Bass kernel (final).

Full-input contract: kernel(**inputs) takes the unsharded inputs from
setup_inputs() and returns the full [8, 4, 64, 62, 1] output.

Sharding: pure data parallel over the leading batch dim B=8 -> one batch
sample per NeuronCore.

Structure:
  - The SRA routing decision is computed on host (numpy) per batch sample,
    like the host-side weight folding the baseline already did.  The flag
    only selects the (s, lp) vs (lp, s) token order, so it is folded into
    the host-side transpose of x: the device program is flag-independent
    and identical on all 8 cores.
  - x is pre-transposed on host to [N1, P, S*LP] token-major layout and
    cast to bf16; no PE transposes, no DVE casts, single-select mm1.
  - All matmuls bf16 (512 moving cols, K=M=128) accumulating fp32 PSUM.
  - Scalar-engine silu is the secondary bottleneck (8-core P0 clock), so
    mm1 and conv both batch PAIRS of 512-col tiles into 2-bank PSUM tiles
    drained by a single [128, 1024] activation (same per-partition bias).
  - The final (W2 @ Wr)-folded projection: 4 concurrent M=1 matmuls on 4
    distinct PE column groups of one PSUM bank, reduced by a 4-op DVE
    chain (one PSUM operand per op).
  - hbuf is double-buffered across n; mm1(n+1) pair-units weave into
    conv(n)'s matmul stream (no PE idle gap at n boundaries, HAM warm).
  - A post-Tile IR pass removes redundant consecutive same-weight
    InstLdweights (the PE keeps the stationary operand loaded), removing
    ~430 exposed weight-reload slots from the tensor queue.
  - b2@Wr + br is a scalar constant added on host at the end.

Measured on 8 NeuronCores: 364.9us (baseline) -> 237.7us, rel err 3.6e-3.
"""

import contextlib

import numpy as np

import concourse.bass as bass
import concourse.tile as tile
from concourse import bacc, mybir

# Problem shapes (hardcoded per contract)
B = 8
N1, S, L, P, F = 4, 64, 8192, 128, 256
LP = L // 128          # 64 patches per series
FH = 128               # half of F (PE partition limit)
CB = 512               # matmul moving-dim batch (columns)
NB = (S * LP) // CB    # 8 batches of 512 cols per n
NP = NB // 2           # 4 batch-pairs per n
OUTL = LP - 2          # 62 valid conv outputs per patch-block
NCORES = 8
NTOK = S * LP          # 4096 tokens per n

F32 = mybir.dt.float32
BF16 = mybir.dt.bfloat16
ALU = mybir.AluOpType
ACTF = mybir.ActivationFunctionType


def _dedup_ldweights(nc):
    """Remove InstLdweights whose weight AP matches the immediately previous
    PE-queue InstLdweights with only non-transpose InstMatmult in between.
    The PE keeps the stationary operand loaded across matmuls, so a reload
    of identical weights is a no-op that can cost an exposed queue slot."""
    removed = 0
    for f in nc.m.functions:
        for blk in f.blocks:
            prev_key = None
            to_remove = []
            for inst in blk.instructions:
                tn = type(inst).__name__
                if getattr(inst, "engine", None) != mybir.EngineType.PE:
                    continue
                if tn == "InstLdweights":
                    key = (
                        str(inst.ins[0]),
                        str(getattr(inst, "perf_mode", None)),
                        str(getattr(inst, "is_transpose", None)),
                        str(getattr(inst, "tile_position", None)),
                    )
                    if key == prev_key:
                        to_remove.append(inst)
                    else:
                        prev_key = key
                elif tn == "InstMatmult":
                    if getattr(inst, "is_transpose", False):
                        prev_key = None
                else:
                    # any other PE instruction: conservatively reset
                    prev_key = None
            for inst in to_remove:
                blk.instructions.remove(inst)
                removed += 1
    return removed


def build_program():
    nc = bacc.Bacc("TRN2", target_bir_lowering=False, debug=False)

    x_d = nc.dram_tensor("x", [N1, P, NTOK], BF16, kind="ExternalInput")
    w1_d = nc.dram_tensor("w1", [P, 2, 2, FH], BF16, kind="ExternalInput")
    cwt_d = nc.dram_tensor("cwt", [FH, 2, 3, 2, 2, FH], BF16, kind="ExternalInput")
    w2p_d = nc.dram_tensor("w2p", [P, 4], BF16, kind="ExternalInput")
    bp_d = nc.dram_tensor("biasp", [P, 8], F32, kind="ExternalInput")
    out_d = nc.dram_tensor("out", [N1, S, OUTL], F32, kind="ExternalOutput")

    with tile.TileContext(nc) as tc:
        with contextlib.ExitStack() as ctx:
            _build_body(nc, tc, ctx, x_d, w1_d, cwt_d, w2p_d, bp_d, out_d)
    _dedup_ldweights(nc)
    nc.compile()
    return nc


def _build_body(nc, tc, ctx, x_d, w1_d, cwt_d, w2p_d, bp_d, out_d):
    const = ctx.enter_context(tc.tile_pool(name="const", bufs=1))

    # ---- resident weights (bf16, pre-packed on host) ----------------------
    # w1 rides the sync queue FIRST (the very first matmul needs it);
    # the rest load on the scalar queue in parallel.
    w1_sb = const.tile([P, 2, 2, FH], BF16)
    nc.sync.dma_start(out=w1_sb, in_=w1_d.ap())
    bp_sb = const.tile([P, 8], F32)
    nc.scalar.dma_start(out=bp_sb, in_=bp_d.ap())
    cwt_sb = const.tile([FH, 2, 3, 2, 2, FH], BF16)
    nc.scalar.dma_start(out=cwt_sb, in_=cwt_d.ap())
    w2p_sb = const.tile([P, 4], BF16)
    nc.scalar.dma_start(out=w2p_sb, in_=w2p_d.ap())

    # ---- persistent buffers ----------------------------------------------
    xt_p = ctx.enter_context(tc.tile_pool(name="xt", bufs=2))
    xts = [None] * (N1 + 1)

    hpool = ctx.enter_context(tc.tile_pool(name="ht", bufs=1))
    # hbuf[(set, d, i)]: [P, NTOK + 2] bf16, 2 zero pad cols for conv tail
    hbuf = {}
    for st in range(2):
        for d in range(2):
            for i in range(2):
                t = hpool.tile([P, NTOK + 2], BF16, name=f"ht_{st}_{d}_{i}")
                nc.vector.memset(t[:, NTOK : NTOK + 2], 0.0)
                hbuf[(st, d, i)] = t

    mm_ps = ctx.enter_context(tc.tile_pool(name="mmps", bufs=1, space="PSUM"))
    cv_ps = ctx.enter_context(tc.tile_pool(name="cvps", bufs=2, space="PSUM"))
    dt_ps = ctx.enter_context(tc.tile_pool(name="dtps", bufs=2, space="PSUM"))
    sff_p = ctx.enter_context(tc.tile_pool(name="sff", bufs=4))
    sfb_p = ctx.enter_context(tc.tile_pool(name="sfb", bufs=2))
    tt_p = ctx.enter_context(tc.tile_pool(name="tt", bufs=2))
    os_p = ctx.enter_context(tc.tile_pool(name="osb", bufs=2))

    outs = [None] * N1
    sffp = {}  # b-pair index -> sf pair tile

    def xt_dma(n, nchunks=2):
        xts[n] = xt_p.tile([P, NTOK], BF16, name=f"xt{n}", tag="xt")
        step = NTOK // nchunks
        for c in range(nchunks):
            nc.sync.dma_start(
                out=xts[n][:, c * step : (c + 1) * step],
                in_=x_d.ap()[n][:, c * step : (c + 1) * step],
            )

    def mm1_pair(n, bp, d, i, pool=None):
        """h[d,i][:, 1024bp:1024(bp+1)] = silu(W1[d,i]^T @ xT + b1).

        Two same-weight matmuls into a 2-bank PSUM tile, one ACT drain."""
        ps = (pool or mm_ps).tile([P, 2, CB], F32)
        for j in range(2):
            nc.tensor.matmul(
                out=ps[:, j, :],
                lhsT=w1_sb[:, d, i, :],
                rhs=xts[n][:, CB * (2 * bp + j) : CB * (2 * bp + j + 1)],
                start=True,
                stop=True,
                skip_group_check=True,
            )
        nc.scalar.activation(
            out=hbuf[(n % 2, d, i)][:, 2 * CB * bp : 2 * CB * (bp + 1)],
            in_=ps,
            func=ACTF.Silu,
            bias=bp_sb[:, 2 * d + i : 2 * d + i + 1],
            scale=1.0,
        )

    def mm1_pairs(n):
        for bp in range(NP):
            for d in range(2):
                for i in range(2):
                    yield (n, bp, d, i)

    def conv_opass(n, d, bp, o):
        """One o-half of a conv bi-pair: 12 matmuls (6 weights x 2 bi) into
        a 2-bank PSUM tile; returns it for the ACT drain."""
        ps = cv_ps.tile([P, 2, CB], F32)
        for idx, (i, k) in enumerate([(i, k) for i in range(2) for k in range(3)]):
            for j in range(2):
                nc.tensor.matmul(
                    out=ps[:, j, :],
                    lhsT=cwt_sb[:, d, k, i, o, :],
                    rhs=hbuf[(n % 2, d, i)][
                        :, CB * (2 * bp + j) + k : CB * (2 * bp + j) + k + CB
                    ],
                    start=(idx == 0),
                    stop=(idx == 5),
                    skip_group_check=True,
                )
        return ps

    def conv_act(n, d, o, ps, sfp):
        nc.scalar.activation(
            out=sfp[:, o, :, :],
            in_=ps,
            func=ACTF.Silu,
            bias=bp_sb[:, 4 + 2 * d + o : 5 + 2 * d + o],
            scale=1.0,
        )

    def conv_pair(n, d, bp, pool, weave=()):
        """Conv bi-pair (2bp, 2bp+1): o=0 12 MMs + ACT, o=1 12 MMs + ACT.
        `weave` holds mm1 pair-units spread around the o-runs."""
        sfp = pool.tile([P, 2, 2, CB], BF16)
        weave = list(weave)
        if weave:
            mm1_pair(*weave.pop(0))
        ps0 = conv_opass(n, d, bp, 0)
        if weave:
            mm1_pair(*weave.pop(0))
        ps1 = conv_opass(n, d, bp, 1)
        conv_act(n, d, 0, ps0, sfp)
        conv_act(n, d, 1, ps1, sfp)
        for u in weave:
            mm1_pair(*u)
        return sfp

    def flip_oj(t, o, j):
        """sf pair tile [P, 2, 2, CB] -> [P, 512] view of (o, j) with its 8
        64-col s-chunks reversed (the bwd direction's S flip)."""
        a = t[:]
        return bass.AP(
            tensor=a.tensor,
            offset=a.offset + (2 * o + j) * CB + 7 * LP,
            ap=[a.ap[0], [-LP, 8], [1, LP]],
        )

    def dot_block(n, b, sfbp, jb, act_copy=False):
        """Folded (W2 @ Wr) projection for output block b: 4 concurrent M=1
        matmuls on distinct PE column groups, then a 4-op reduce chain (one
        PSUM operand per instruction; first op on ScalarE when the DVE is
        the local bottleneck)."""
        sfft = sffp[b // 2]
        jf = b % 2
        dt = dt_ps.tile([P, CB], F32)
        nc.tensor.matmul(
            out=dt[0:1, :], lhsT=w2p_sb[:, 0:1], rhs=sfft[:, 0, jf, :],
            start=True, stop=True, skip_group_check=True,
        )
        nc.tensor.matmul(
            out=dt[32:33, :], lhsT=w2p_sb[:, 1:2], rhs=sfft[:, 1, jf, :],
            start=True, stop=True, skip_group_check=True,
        )
        nc.tensor.matmul(
            out=dt[64:65, :], lhsT=w2p_sb[:, 2:3], rhs=flip_oj(sfbp, 0, jb),
            start=True, stop=True, skip_group_check=True,
        )
        nc.tensor.matmul(
            out=dt[96:97, :], lhsT=w2p_sb[:, 3:4], rhs=flip_oj(sfbp, 1, jb),
            start=True, stop=True, skip_group_check=True,
            tile_position=(0, 96),
        )
        t1 = tt_p.tile([1, CB], F32)
        if act_copy:
            nc.scalar.activation(out=t1, in_=dt[0:1, :], func=ACTF.Copy, bias=0.0)
        else:
            nc.vector.tensor_copy(out=t1, in_=dt[0:1, :])
        t2 = tt_p.tile([1, CB], F32)
        nc.vector.tensor_tensor(out=t2, in0=t1, in1=dt[32:33, :], op=ALU.add)
        t3 = tt_p.tile([1, CB], F32)
        nc.vector.tensor_tensor(out=t3, in0=t2, in1=dt[64:65, :], op=ALU.add)
        nc.vector.tensor_tensor(
            out=outs[n][:, CB * b : CB * (b + 1)],
            in0=t3,
            in1=dt[96:97, :],
            op=ALU.add,
        )

    def out_dma(n):
        ov = outs[n][:].rearrange("q (s l) -> q s l", l=LP)[:, :, 0:OUTL]
        nc.sync.dma_start(out=out_d.ap()[n], in_=ov)

    def take(it, k):
        got = []
        for _ in range(k):
            u = next(it, None)
            if u is not None:
                got.append(u)
        return got

    def d1_phase(n, units, per_block):
        """conv-d1 pair-blocks p=3..0; dots one pair-block late."""
        sfb_tiles = {}
        for idx, p in enumerate(range(NP - 1, -1, -1)):
            nw = per_block[idx] if isinstance(per_block, list) else per_block
            sfb_tiles[p] = conv_pair(n, 1, p, sfb_p, take(units, nw))
            if idx >= 1:
                pq = NP - idx  # sfb pair emitted one block ago
                sp = sfb_tiles.pop(pq)
                b0 = 2 * (NP - 1 - pq)  # dot b indices for sfb pair pq
                dot_block(n, b0, sp, 1)
                dot_block(n, b0 + 1, sp, 0)
        sp = sfb_tiles.pop(0)
        dot_block(n, 2 * NP - 2, sp, 1)
        dot_block(n, 2 * NP - 1, sp, 0)
        out_dma(n)

    # ---- main schedule ----------------------------------------------------
    xt_dma(0, nchunks=4)
    xt_dma(1, nchunks=1)

    # n=0: mm1(0) woven with conv-d0 by readiness (startup is ACT-bound).
    # conv-d0 pair-block K reads d0 hbuf pairs <= K+1; conv-d1 pair-block p
    # (emitted descending) reads d1 pairs p and p+1.
    outs[0] = os_p.tile([1, NTOK], F32, name="outs0", tag="outs")
    for idx, (i, bp) in enumerate([(0, 0), (0, 1), (1, 0), (1, 1)]):
        mm1_pair(0, bp, 0, i, pool=(cv_ps if idx % 2 else mm_ps))
    w0 = {
        0: [(0, 2, 0, 0), (0, 2, 0, 1), (0, 3, 0, 0), (0, 3, 0, 1)],
        1: [(0, 3, 1, 0), (0, 3, 1, 1), (0, 2, 1, 0), (0, 2, 1, 1)],
        2: [(0, 1, 1, 0), (0, 1, 1, 1), (0, 0, 1, 0), (0, 0, 1, 1)],
    }
    u1 = mm1_pairs(1)
    for bp in range(NP):
        wv = w0[bp] if bp in w0 else take(u1, 4)
        sffp[bp] = conv_pair(0, 0, bp, sff_p, wv)
    # remaining 12 mm1(1) pair-units weave into n=0's d1 phase; the last
    # block carries only lead/mid slots so no unit trails into the n=1
    # boundary (its ACT would stall n=1's first weave matmul on mm_ps)
    d1_phase(0, u1, [3, 3, 4, 2])

    # n = 1..3: interleaved segments [conv-d0 q | dots(q-1) | conv-d1 3-q];
    # dot pair q needs d0 pair q and d1 pair 3-q, both from segment q, and
    # is emitted mid-segment q+1 so its rhs ACTs have a full block of slack.
    for n in range(1, N1):
        outs[n] = os_p.tile([1, NTOK], F32, name=f"outs{n}", tag="outs")
        if n + 1 < N1:
            xt_dma(n + 1)
            units = mm1_pairs(n + 1)
        else:
            units = iter(())
        sfb_tiles = {}
        for q in range(NP):
            last_seg = q == NP - 1
            sffp[q] = conv_pair(n, 0, q, sff_p, take(units, 4 if last_seg else 2))
            if q >= 1:
                sp = sfb_tiles.pop(NP - q)
                dot_block(n, 2 * (q - 1), sp, 1)
                dot_block(n, 2 * q - 1, sp, 0)
            sfb_tiles[NP - 1 - q] = conv_pair(
                n, 1, NP - 1 - q, sfb_p, take(units, 0 if last_seg else 2)
            )
        sp = sfb_tiles.pop(0)
        last = n == N1 - 1
        dot_block(n, 2 * NP - 2, sp, 1, act_copy=last)
        dot_block(n, 2 * NP - 1, sp, 0, act_copy=last)
        out_dma(n)


_PROGRAM = None


def _get_program():
    global _PROGRAM
    if _PROGRAM is None:
        _PROGRAM = build_program()
    return _PROGRAM


def _decide(corr):
    """Vectorized SRA_Decider on host: bool [B]."""
    c = np.asarray(corr, np.float64)
    n = c.shape[-1]
    mean = c.mean(axis=-1, keepdims=True)
    std = c.std(axis=-1, ddof=1, keepdims=True)
    norm = (c - mean) / std
    g = np.einsum("bsl,btl->bst", norm, norm) / n
    s = g.shape[-1]
    idx = np.arange(s)
    g[:, idx, idx] = 0.0
    cnt_thr = (g > 0.6).sum(axis=(1, 2)).astype(np.float64)
    cnt_pos = (g > 0.0).sum(axis=(1, 2)).astype(np.float64)
    ratio = np.where(cnt_pos > 0, cnt_thr / np.maximum(cnt_pos, 1.0), 0.0)
    return ratio >= 0.4


def _bf16(a):
    import ml_dtypes

    return np.asarray(a, np.float32).astype(ml_dtypes.bfloat16)


def _pack_weights(inputs):
    f32 = np.float32
    w1 = np.stack(
        [np.asarray(inputs["W1f"], f32), np.asarray(inputs["W1b"], f32)], axis=1
    ).reshape(P, 2, 2, FH)  # [p, d, i, fo]
    cwt = np.empty((2, 3, 2, 2, FH, FH), f32)
    for d, key in enumerate(["Cwf", "Cwb"]):
        cw = np.asarray(inputs[key], f32)  # [F_out, F_in, 3]
        t = np.transpose(cw, (1, 0, 2))  # [fi, fo, k]
        for k in range(3):
            for i in range(2):
                for o in range(2):
                    cwt[d, k, i, o] = t[
                        i * FH : (i + 1) * FH, o * FH : (o + 1) * FH, k
                    ]
    cwt = np.ascontiguousarray(np.transpose(cwt, (4, 0, 1, 2, 3, 5)))
    wr = np.asarray(inputs["Wr"], f32)  # [F, 1]
    w2pf = np.asarray(inputs["W2f"], f32) @ wr  # [F, 1]
    w2pb = np.asarray(inputs["W2b"], f32) @ wr
    w2p = np.stack(
        [w2pf[:FH, 0], w2pf[FH:, 0], w2pb[:FH, 0], w2pb[FH:, 0]], axis=1
    )  # [P, 4]
    cconst = (
        np.asarray(inputs["b2f"], f32) @ wr
        + np.asarray(inputs["b2b"], f32) @ wr
        + np.asarray(inputs["br"], f32)
    ).item()
    bp = np.zeros((P, 8), f32)
    b1f = np.asarray(inputs["b1f"], f32)
    b1b = np.asarray(inputs["b1b"], f32)
    cbf = np.asarray(inputs["Cbf"], f32)
    cbb = np.asarray(inputs["Cbb"], f32)
    bp[:, 0] = b1f[:FH]
    bp[:, 1] = b1f[FH:]
    bp[:, 2] = b1b[:FH]
    bp[:, 3] = b1b[FH:]
    bp[:, 4] = cbf[:FH]
    bp[:, 5] = cbf[FH:]
    bp[:, 6] = cbb[:FH]
    bp[:, 7] = cbb[FH:]
    return _bf16(w1), _bf16(cwt), _bf16(w2p), bp, cconst


def make_in_maps(inputs):
    flags = _decide(np.asarray(inputs["correlations"], np.float32))
    xb = _bf16(inputs["x"]).reshape(B, N1, S, LP, P)
    w1, cwt, w2p, bp, cconst = _pack_weights(inputs)
    in_maps = []
    for b in range(NCORES):
        if flags[b]:
            # channel_mixing: token (i, j) = x[b, n, j, i*128:(i+1)*128]
            xt = np.transpose(xb[b], (0, 3, 2, 1))
        else:
            # channel_independent: token (i, j) = x[b, n, i, j*128:(j+1)*128]
            xt = np.transpose(xb[b], (0, 3, 1, 2))
        xt = np.ascontiguousarray(xt).reshape(N1, P, NTOK)
        in_maps.append({"x": xt, "w1": w1, "cwt": cwt, "w2p": w2p, "biasp": bp})
    return in_maps, cconst


def kernel(**inputs) -> np.ndarray:
    from concourse.bass_utils import run_bass_kernel_spmd

    nc = _get_program()
    in_maps, cconst = make_in_maps(inputs)
    res = run_bass_kernel_spmd(nc, in_maps, core_ids=list(range(NCORES)))
    out = np.stack([res.results[b]["out"] for b in range(NCORES)])
    return (out + cconst)[..., None].astype(np.float32)  # [8, 4, 64, 62, 1]
